# revision 33
# baseline (speedup 1.0000x reference)
"""AttentionHead kernel for 8 TRN2 NeuronCores.

Problem: q,k,v [4, 2048, 1024] f32; Wq/Wk/Wv [1024, 128]; out = softmax(
(qWq)(kWk)^T / sqrt(128)) @ (vWv)  -> [4, 2048, 128].

Sharding: core c = 2b+j owns batch b and query rows [1024j, 1024j+1024).
K and V for the WHOLE batch-b sequence are shipped to both cores of the
pair (host-side duplication), so no collectives are needed at all.  All
inputs are pre-transposed on the host into [hidden, seq] layout, so the
kernel never transposes activations on the PE: projections read hT-major
tiles directly as matmul operands.

On-chip dataflow (per core):
  QT [d, sq]  = Wq-chunk.T @ qT-chunk   (PSUM accum over 8 h-chunks)
  KT [d, sk]  = Wk-chunk.T @ kT-chunk
  value [sk, d] = vT-chunk.T @ Wv-chunk + bv (ones-row x bv-row matmul)
  scoresT[sk-chunk] = KT-cols.T @ QT    -> [128, 1024] PSUM
  ex = exp(scores * 1/sqrt(d))          (ACT, bf16, unnormalized)
  rowsum: DVE running accumulator over ex chunks (bf16), then one GPSIMD
          partition_all_reduce -> broadcast row (PE-free)
  ctxT[d, sq] += value-chunk.T @ ex     two seg-wide PSUM accumulation
          groups (one bank + one tile per sq-half; PSUM allows a single
          live accumulation group per bank) interleaved into the chunk
          loop at a 2-chunk lag, then evicted, transposed back to
          [sq, d] on the PE and normalized with recip(rowsum).
The rowsum row is relaid to per-partition scalars with 8 tiny PE
transposes (no DRAM round-trip).  K-proj / V-proj segments are
interleaved into the attention chunk loop so the PE consumes each DMA
segment right after it lands.  Tiles touched by two engines are split
per engine: the dependency tracker serializes cross-engine accesses at
tile granularity.
"""

import os
from contextlib import ExitStack

# The kernel needs jax's axon TRN2 backend; a pinned cpu-only platform list
# (used by some harnesses for the jax reference) would hide the devices.
if os.environ.get("JAX_PLATFORMS") not in (None, "", "axon"):
    del os.environ["JAX_PLATFORMS"]

import numpy as np

import concourse.bass as bass
import concourse.bass_isa as bass_isa
import concourse.tile as tile
import concourse.mybir as mybir
from concourse import bacc
from concourse.bass_utils import run_bass_kernel_spmd
from concourse.masks import make_identity

B, S, H, D = 4, 2048, 1024, 128
N_CORES = 8
SQ = 1024  # query rows per core
SK = 2048  # kv rows per batch (full sequence)
HC = H // 128  # 8 chunks of the hidden dim
NCK = SK // 128  # 16 sk chunks
NG = SQ // 128  # 8 sq chunks
F32 = mybir.dt.float32
BF16 = mybir.dt.bfloat16
SCALE = 1.0 / float(np.sqrt(np.float32(D)))
WARM1 = 52  # identity transposes covering the first DMAs latency
WARM2 = 0  # gaps under ~3us do not reset the PE ramp; filler would only delay

_NC_CACHE = {}


def build():
    nc = bacc.Bacc(None, target_bir_lowering=False)
    qt_d = nc.declare_dram_parameter("qt", [H, SQ], BF16, isOutput=False)
    kt_d = nc.declare_dram_parameter("kt", [H, SK], BF16, isOutput=False)
    vt_d = nc.declare_dram_parameter("vt", [H, SK], BF16, isOutput=False)
    # weights pre-packed on host to [128, H] (partition-major chunks)
    w_d = {
        n: nc.declare_dram_parameter(f"w{n}", [128, H], BF16, isOutput=False)
        for n in ("q", "k", "v")
    }
    bqk_d = nc.declare_dram_parameter("bqk", [128, 2], F32, isOutput=False)
    bv_d = nc.declare_dram_parameter("bv", [1, D], F32, isOutput=False)
    out_d = nc.declare_dram_parameter("out", [SQ, D], F32, isOutput=True)

    with tile.TileContext(nc) as tc, ExitStack() as top:
        const = top.enter_context(tc.tile_pool(name="const", bufs=1))
        identity = const.tile([128, 128], F32)
        make_identity(nc, identity)
        identity_b = const.tile([128, 128], BF16)
        nc.vector.tensor_copy(out=identity_b[:], in_=identity[:])
        ones_b = const.tile([1, 128], BF16)
        nc.vector.memset(ones_b[:], 1.0)
        # preload the ACT exp table during the DMA-bound prologue
        dummy = const.tile([1, 1], F32)
        nc.vector.memset(dummy[:], 0.0)
        nc.scalar.activation(
            dummy[:], dummy[:], mybir.ActivationFunctionType.Exp
        )

        # DMA issue order = arrival order (the DMA engines serialize):
        # biases/Wq/q first so Q-proj starts early, then alternate kt/vt
        # segments to pace the interleaved proj/attention loop.
        bqk_sb = const.tile([128, 2], F32)
        nc.sync.dma_start(out=bqk_sb[:], in_=bqk_d[:])
        bv_f = const.tile([1, D], F32)
        nc.sync.dma_start(out=bv_f[:], in_=bv_d[:])
        bv_b = const.tile([1, D], BF16)
        nc.vector.tensor_copy(out=bv_b[:], in_=bv_f[:])

        stage = top.enter_context(tc.tile_pool(name="stage", bufs=1))
        qt_in = stage.tile([128, HC, SQ], BF16)
        kt_in = stage.tile([128, HC, SK], BF16)
        vt_in = stage.tile([128, HC, SK], BF16)
        qt_view = qt_d[:].rearrange("(c p) s -> p c s", p=128)
        kt_view = kt_d[:].rearrange("(c p) s -> p c s", p=128)
        vt_view = vt_d[:].rearrange("(c p) s -> p c s", p=128)

        w_sb = {}

        def dma_w(n):
            w_sb[n] = const.tile([128, H], BF16, name=f"w{n}_sb")
            nc.sync.dma_start(out=w_sb[n][:], in_=w_d[n][:])

        def dma_cols(dest, view, lo, hi):
            nc.sync.dma_start(
                out=dest[:, :, lo:hi], in_=view[:, :, lo:hi]
            )

        def dma_seg(dest, view, i):
            dma_cols(dest, view, i * 512, (i + 1) * 512)

        dma_w("q")
        dma_seg(qt_in, qt_view, 0)
        dma_seg(qt_in, qt_view, 1)
        dma_w("k")
        dma_seg(kt_in, kt_view, 0)
        dma_w("v")
        dma_seg(vt_in, vt_view, 0)
        for i in range(1, 4):
            dma_seg(kt_in, kt_view, i)
            dma_seg(vt_in, vt_view, i)

        proj = top.enter_context(tc.tile_pool(name="proj", bufs=1))
        qt_sb = proj.tile([128, SQ], BF16)
        # per-segment tiles: a single big tile would serialize the DVE
        # eviction writes against unrelated in-flight PE reads
        kt_segs = [
            proj.tile([128, 512], BF16, name=f"kt_sb{i}") for i in range(4)
        ]
        value_segs = [
            proj.tile([128, 512], BF16, name=f"value_sb{i}") for i in range(4)
        ]

        fin = top.enter_context(tc.tile_pool(name="fin", bufs=1))
        recip = fin.tile([128, NG], F32)
        # separate eviction tiles per engine: cross-engine writes into one
        # tile get serialized by the dependency tracker
        out_a = fin.tile([128, NG // 2, D], F32)  # DVE half (g0..3)
        out_b = fin.tile([128, NG // 2, D], F32)  # ACT half (g4..7)
        rs_bc = fin.tile([128, SQ], F32)  # rowsum broadcast over partitions

        sc_ps = top.enter_context(tc.tile_pool(name="sc_ps", bufs=2, space="PSUM"))
        pj_ps = top.enter_context(tc.tile_pool(name="pj_ps", bufs=2, space="PSUM"))
        ctx_ps = top.enter_context(tc.tile_pool(name="ctx_ps", bufs=1, space="PSUM"))
        att = top.enter_context(tc.tile_pool(name="att", bufs=NCK))
        acc_pool = top.enter_context(tc.tile_pool(name="acc", bufs=1))

        # ---- PE warm-up: keep the PE busy during the initial DMAs so the
        # p-state ramp completes before the first real matmul ----
        warm_ps = pj_ps.tile([128, 512], BF16, tag="pj")

        def warmup(n):
            for i in range(n):
                nc.tensor.transpose(
                    warm_ps[:, (i % 4) * 128 : (i % 4 + 1) * 128],
                    identity_b[:],
                    identity_b[:],
                )

        warmup(WARM1)

        def qk_proj_cols(name, bias_ap, lo, hi):
            """proj columns [lo, hi) of (W.T @ xT) + bias."""
            src_in = qt_in if name == "q" else kt_in
            pj = pj_ps.tile(
                [128, hi - lo], F32, tag="pj", name=f"pj_{name}{lo}"
            )
            for c in range(HC):
                nc.tensor.matmul(
                    pj[:],
                    w_sb[name][:, c * 128 : (c + 1) * 128],
                    src_in[:, c, lo:hi],
                    start=(c == 0),
                    stop=(c == HC - 1),
                )
            if name == "q":
                dest = qt_sb[:, lo:hi]
            else:
                dest = kt_segs[lo // 512][:, lo % 512 : lo % 512 + hi - lo]
            nc.vector.tensor_scalar(
                out=dest,
                in0=pj[:],
                scalar1=bias_ap,
                scalar2=None,
                op0=mybir.AluOpType.add,
            )

        def qk_proj_seg(name, dest_sb, src_in, bias_ap, seg):
            qk_proj_cols(name, bias_ap, seg * 512, (seg + 1) * 512)

        def v_proj_seg(seg):
            """value_sb[:, seg*512:+512] <- 4 chunks of vT-chunk.T@Wv + bv."""
            vp = pj_ps.tile([128, 512], F32, tag="pj", name=f"vp{seg}")
            for cc in range(4):
                skc = seg * 4 + cc
                # bias row: ones-col x bv-row (opens the accumulation group)
                nc.tensor.matmul(
                    vp[:, cc * 128 : (cc + 1) * 128],
                    ones_b[:],
                    bv_b[:],
                    start=True,
                    stop=False,
                )
                for c in range(HC):
                    nc.tensor.matmul(
                        vp[:, cc * 128 : (cc + 1) * 128],
                        vt_in[:, c, skc * 128 : (skc + 1) * 128],
                        w_sb["v"][:, c * 128 : (c + 1) * 128],
                        start=False,
                        stop=(c == HC - 1),
                    )
            nc.vector.tensor_copy(out=value_segs[seg][:], in_=vp[:])

        # ---- Q projection (both segments) ----
        for seg in range(2):
            qk_proj_seg("q", None, qt_in, bqk_sb[:, 0:1], seg)
        warmup(WARM2)

        # ---- interleaved K/V projection + attention ----
        exs = []
        ctx_tiles = {}

        def scores_chunk(c):
            sc = sc_ps.tile([128, SQ], F32, tag="sc", name=f"sc{c}")
            for seg in range(2):
                nc.tensor.matmul(
                    sc[:, seg * 512 : (seg + 1) * 512],
                    kt_segs[c // 4][:, (c % 4) * 128 : (c % 4 + 1) * 128],
                    qt_sb[:, seg * 512 : (seg + 1) * 512],
                    start=True,
                    stop=True,
                )
            ex = att.tile([128, SQ], BF16, tag="ex", name=f"ex{c}")
            nc.scalar.activation(
                ex[:], sc[:], mybir.ActivationFunctionType.Exp, scale=SCALE
            )
            exs.append(ex)

        # rowsum: bf16 running accumulator on the DVE
        acc_tile = [None]

        def rs_accum(c):
            """Fold exp chunk c into the running rowsum accumulator."""
            if c == 0:
                return
            if acc_tile[0] is None:
                acc_tile[0] = acc_pool.tile(
                    [128, SQ], BF16, tag="acc", name="rs_acc"
                )
                nc.vector.tensor_tensor(
                    out=acc_tile[0][:], in0=exs[0][:], in1=exs[1][:],
                    op=mybir.AluOpType.add,
                )
            else:
                nc.vector.tensor_tensor(
                    out=acc_tile[0][:], in0=acc_tile[0][:], in1=exs[c][:],
                    op=mybir.AluOpType.add,
                )

        # contextT [d, sq]: two seg-wide accumulation groups, one PSUM bank
        # (and one tile -- separate tiles keep the cross-engine eviction
        # reads independent) per sq-half, living across the whole chunk loop
        ctxT = [
            ctx_ps.tile([128, 512], F32, tag=f"ctx{seg}", name=f"ctxT{seg}")
            for seg in range(2)
        ]

        def ctx_chunk(c):
            for seg in range(2):
                nc.tensor.matmul(
                    ctxT[seg][:],
                    value_segs[c // 4][:, (c % 4) * 128 : (c % 4 + 1) * 128],
                    exs[c][:, seg * 512 : (seg + 1) * 512],
                    start=(c == 0),
                    stop=(c == NCK - 1),
                )

        for i in range(4):
            qk_proj_seg("k", None, kt_in, bqk_sb[:, 1:2], i)
            scores_chunk(4 * i)
            scores_chunk(4 * i + 1)
            if i < 3:
                v_proj_seg(i)
                scores_chunk(4 * i + 2)
                scores_chunk(4 * i + 3)
            else:
                scores_chunk(4 * i + 2)
                scores_chunk(4 * i + 3)
                v_proj_seg(i)
            for c in range(max(0, 4 * i - 2), 4 * i + 2):
                ctx_chunk(c)
                rs_accum(c)
        for c in (NCK - 2, NCK - 1):
            ctx_chunk(c)
            rs_accum(c)

        # final rowsum: one partition all-reduce (values upcast to f32)
        nc.gpsimd.partition_all_reduce(
            out_ap=rs_bc[:],
            in_ap=acc_tile[0][:],
            channels=128,
            reduce_op=bass_isa.ReduceOp.add,
        )

        # ---- tail: evict ctxT, transpose back to [sq, d], normalize ----
        out_view = out_d[:].rearrange("(j p) d -> p j d", p=128)
        ct_a = fin.tile([128, 512], BF16)
        ct_b = fin.tile([128, 512], BF16)
        nc.vector.tensor_copy(out=ct_a[:], in_=ctxT[0][:])
        nc.scalar.activation(
            ct_b[:], ctxT[1][:], mybir.ActivationFunctionType.Copy
        )
        ctT_a = ctx_ps.tile([128, NG // 2, D], BF16, tag="ctx0", name="ctT_a")
        ctT_b = pj_ps.tile([128, NG // 2, D], BF16, tag="pj", name="ctT_b")
        for g in range(NG):
            src_sb = ct_a if g < 4 else ct_b
            dst_ps = ctT_a if g < 4 else ctT_b
            nc.tensor.transpose(
                dst_ps[:, g % 4, :],
                src_sb[:, (g % 4) * 128 : (g % 4 + 1) * 128],
                identity_b[:],
            )
        # rowsum row -> per-partition scalars (8 tiny transposes)
        rsT = pj_ps.tile([128, NG], F32, tag="pj")
        for t in range(NG):
            nc.tensor.transpose(
                rsT[:, t : t + 1],
                rs_bc[0:1, t * 128 : (t + 1) * 128],
                identity[0:1, 0:1],
            )
        nc.vector.reciprocal(out=recip[:], in_=rsT[:])

        # normalized evictions (DVE for g0..3, ACT for g4..7) + output DMAs
        for g in range(NG // 2):
            nc.vector.tensor_scalar(
                out=out_a[:, g, :],
                in0=ctT_a[:, g, :],
                scalar1=recip[:, g : g + 1],
                scalar2=None,
                op0=mybir.AluOpType.mult,
            )
            nc.scalar.activation(
                out_b[:, g, :],
                ctT_b[:, g, :],
                mybir.ActivationFunctionType.Copy,
                scale=recip[:, 4 + g : 5 + g],
            )
        nc.sync.dma_start(out=out_view[:, 0:4, :], in_=out_a[:])
        nc.sync.dma_start(out=out_view[:, 4:8, :], in_=out_b[:])

    nc.compile()
    return nc


def _pack_w(w):
    # [H, D] -> [128, H]: W_p[p, c*128 + d] = W[c*128 + p, d]
    return np.ascontiguousarray(
        w.reshape(HC, 128, D).transpose(1, 0, 2).reshape(128, H)
    )


def kernel(q, k, v, Wq, bq, Wk, bk, Wv, bv):
    import ml_dtypes

    bf16 = ml_dtypes.bfloat16
    q = np.asarray(q, dtype=np.float32).astype(bf16)
    k = np.asarray(k, dtype=np.float32).astype(bf16)
    v = np.asarray(v, dtype=np.float32).astype(bf16)
    Wq_p = _pack_w(np.asarray(Wq, dtype=np.float32).astype(bf16))
    Wk_p = _pack_w(np.asarray(Wk, dtype=np.float32).astype(bf16))
    Wv_p = _pack_w(np.asarray(Wv, dtype=np.float32).astype(bf16))
    bqk = np.ascontiguousarray(
        np.stack(
            [np.asarray(bq, np.float32), np.asarray(bk, np.float32)], axis=1
        )
    )
    bv_row = np.ascontiguousarray(np.asarray(bv, np.float32)[None, :])

    if "nc" not in _NC_CACHE:
        _NC_CACHE["nc"] = build()
    nc = _NC_CACHE["nc"]

    half = S // 2  # 1024
    # host-side layout prep only (slice / transpose / cast)
    kt_full = [np.ascontiguousarray(k[b].T) for b in range(B)]
    vt_full = [np.ascontiguousarray(v[b].T) for b in range(B)]
    in_maps = []
    for c in range(N_CORES):
        b, j = c // 2, c % 2
        sl = slice(j * half, (j + 1) * half)
        in_maps.append(
            {
                "qt": np.ascontiguousarray(q[b, sl].T),
                "kt": kt_full[b],
                "vt": vt_full[b],
                "wq": Wq_p,
                "wk": Wk_p,
                "wv": Wv_p,
                "bqk": bqk,
                "bv": bv_row,
            }
        )
    res = run_bass_kernel_spmd(nc, in_maps, list(range(N_CORES)))
    out = np.empty((B, S, D), dtype=np.float32)
    for c in range(N_CORES):
        b, j = c // 2, c % 2
        out[b, j * half : (j + 1) * half] = res.results[c]["out"]
    return out
